# revision 10
# baseline (speedup 1.0000x reference)
"""Trainium2 Bass kernel for similarity-matrix penalty (gnn message passing).

penalty = sum_{b,k} S[b,k] * || P[i_b] - P[j_{b,k}] ||_2

Strategy (per the batch-sharding hint):
  - Shard the batch dim B=4096 across 8 cores (512 b's each); P is
    replicated to every core's HBM.
  - Per core, 4 chunks of 128 b's. Chunk layout: partition p <-> b,
    free dim = 64 k-slots x 128 d.
  - ACT/DVE write -P[i_b] broadcast across the 64 k-slots (split to
    balance engine load).
  - One big indirect DMA (SWDGE) gathers the 8192 P[j] rows per chunk
    with CCE accumulate-add, materializing diff = P[j] - P[i] directly.
  - ACT Square pass, then one grouped DVE tensor_reduce gives per-(b,k)
    norm^2.
  - sqrt on ACT, multiply by S, free-dim reduce -> [128,1] per core.
  - Host adds the 8x128 partials.
"""

import numpy as np

import concourse.bass as bass
import concourse.mybir as mybir
import concourse.tile as tile
from concourse import bacc
from concourse import bass_utils

N_ROWS = 500000
D = 128
B = 4096
K = 64
N_CORES = 8
B_PER_CORE = B // N_CORES      # 512
CHUNKS = 4
BC = B_PER_CORE // CHUNKS      # 128 b's per chunk (one per partition)
FP32 = mybir.dt.float32
I32 = mybir.dt.int32

# k-slots of the -P[i] broadcast written by ACT; the rest go to DVE.
KA = 40

_PROGRAM_CACHE = {}


def _build_program(repeat=1):
    nc = bacc.Bacc(
        "TRN2",
        debug=False,
        enable_asserts=False,
        target_bir_lowering=False,
        num_devices=N_CORES,
    )

    P_d = nc.dram_tensor("P", [N_ROWS, D], FP32, kind="ExternalInput")
    idxI_d = nc.dram_tensor("idxI", [128, CHUNKS], I32, kind="ExternalInput")
    idxJ_d = nc.dram_tensor("idxJ", [128, CHUNKS * K], I32, kind="ExternalInput")
    S_d = nc.dram_tensor("S", [128, CHUNKS * K], FP32, kind="ExternalInput")
    out_d = nc.dram_tensor("out", [128, 1], FP32, kind="ExternalOutput")

    with tile.TileContext(nc) as tc:
        with (
            tc.tile_pool(name="persist", bufs=1) as pp,
            tc.tile_pool(name="work", bufs=2) as wp,
            tc.tile_pool(name="small", bufs=2) as sp,
        ):
            idxI_sb = pp.tile([128, CHUNKS], I32)
            nc.sync.dma_start(idxI_sb[:], idxI_d[:, :])
            idxJ_sb = pp.tile([128, CHUNKS * K], I32)
            nc.sync.dma_start(idxJ_sb[:], idxJ_d[:, :])
            S_sb = pp.tile([128, CHUNKS * K], FP32)
            nc.sync.dma_start(S_sb[:], S_d[:, :])

            # Gather the 512 P[i] rows (HW supports one index per partition
            # per indirect DMA): piAll[p, c, :] = P[idxI[p, c]]
            piAll = pp.tile([128, CHUNKS, D], FP32)
            for c in range(CHUNKS):
                nc.gpsimd.indirect_dma_start(
                    out=piAll[:, c, :],
                    out_offset=None,
                    in_=P_d[:, :],
                    in_offset=bass.IndirectOffsetOnAxis(
                        ap=idxI_sb[:, c : c + 1], axis=0
                    ),
                )

            norm2 = pp.tile([128, CHUNKS * K], FP32)

            for _rep in range(repeat):
              for c in range(CHUNKS):
                # big[p, k, :] will hold P[j_{b,k}] - P[i_b] for b = c*128+p
                big = wp.tile([128, K, D], FP32)
                # 1) write -P[i_b] broadcast over the k axis (ACT + DVE split)
                if KA > 0:
                    nc.scalar.mul(
                        out=big[:, :KA, :],
                        in_=piAll[:, c : c + 1, :].to_broadcast((128, KA, D)),
                        mul=-1.0,
                    )
                if KA < K:
                    nc.vector.tensor_scalar_mul(
                        big[:, KA:, :],
                        piAll[:, c : c + 1, :].to_broadcast((128, K - KA, D)),
                        -1.0,
                    )
                # 2) gather the 8192 P[j] rows, accumulating (CCE add).
                #    HW consumes one index per partition per call -> per-k calls.
                for k in range(K):
                    nc.gpsimd.indirect_dma_start(
                        out=big[:, k, :],
                        out_offset=None,
                        in_=P_d[:, :],
                        in_offset=bass.IndirectOffsetOnAxis(
                            ap=idxJ_sb[:, c * K + k : c * K + k + 1], axis=0
                        ),
                        compute_op=mybir.AluOpType.add,
                    )
                # 3) square on ACT, grouped free-dim reduce on DVE
                sq = wp.tile([128, K, D], FP32)
                nc.scalar.square(sq[:], big[:])
                nc.vector.reduce_sum(
                    norm2[:, c * K : (c + 1) * K],
                    sq[:],
                    axis=mybir.AxisListType.X,
                )

            # finals: sqrt -> *S -> free-dim reduce -> DRAM
            norms = pp.tile([128, CHUNKS * K], FP32)
            nc.scalar.sqrt(norms[:], norm2[:])
            weighted = pp.tile([128, CHUNKS * K], FP32)
            nc.vector.tensor_tensor(
                out=weighted[:], in0=norms[:], in1=S_sb[:], op=mybir.AluOpType.mult
            )
            res = pp.tile([128, 1], FP32)
            nc.vector.reduce_sum(res[:], weighted[:], axis=mybir.AxisListType.X)
            nc.sync.dma_start(out_d[:, :], res[:])

    nc.compile()
    return nc


def get_program(repeat=1):
    if repeat not in _PROGRAM_CACHE:
        _PROGRAM_CACHE[repeat] = _build_program(repeat)
    return _PROGRAM_CACHE[repeat]


def make_in_maps(P, i_indices, j_indices, S_vals):
    P = np.ascontiguousarray(np.asarray(P, dtype=np.float32))
    i_idx = np.asarray(i_indices).astype(np.int32)
    j_idx = np.asarray(j_indices).astype(np.int32)
    S = np.asarray(S_vals, dtype=np.float32)
    in_maps = []
    for core in range(N_CORES):
        b0 = core * B_PER_CORE
        i_c = i_idx[b0 : b0 + B_PER_CORE]            # [512]
        j_c = j_idx[b0 : b0 + B_PER_CORE]            # [512, 64]
        S_c = S[b0 : b0 + B_PER_CORE]                # [512, 64]
        idxI = np.ascontiguousarray(i_c.reshape(CHUNKS, BC).T)          # [128, 4]
        idxJ = np.ascontiguousarray(
            j_c.reshape(CHUNKS, BC, K).transpose(1, 0, 2).reshape(BC, CHUNKS * K)
        )
        S_arr = np.ascontiguousarray(
            S_c.reshape(CHUNKS, BC, K).transpose(1, 0, 2).reshape(BC, CHUNKS * K)
        )
        in_maps.append({"P": P, "idxI": idxI, "idxJ": idxJ, "S": S_arr})
    return in_maps


def run_hw(in_maps, trace=False, repeat=1):
    nc = get_program(repeat)
    return bass_utils.run_bass_kernel_spmd(
        nc,
        in_maps,
        core_ids=list(range(N_CORES)),
        trace=trace,
    )


def kernel(P, i_indices, j_indices, S_vals):
    in_maps = make_in_maps(P, i_indices, j_indices, S_vals)
    res = run_hw(in_maps, trace=False)
    total = 0.0
    for core in range(N_CORES):
        total += float(np.asarray(res.results[core]["out"], dtype=np.float64).sum())
    return np.float32(total)


# revision 12
# speedup vs baseline: 1.2737x; 1.2737x over previous
"""Trainium2 Bass kernel for similarity-matrix penalty (gnn message passing).

penalty = sum_{b,k} S[b,k] * || P[i_b] - P[j_{b,k}] ||_2

Strategy (per the batch-sharding hint):
  - Shard the batch dim B=4096 across 8 cores (512 b's each); P is
    replicated to every core's HBM.
  - Per core, 4 chunks of 128 b's. Chunk layout: partition p <-> b,
    free dim = 64 k-slots x 128 d.
  - ACT/DVE write -P[i_b] broadcast across the 64 k-slots (split to
    balance engine load).
  - One big indirect DMA (SWDGE) gathers the 8192 P[j] rows per chunk
    with CCE accumulate-add, materializing diff = P[j] - P[i] directly.
  - ACT Square pass, then one grouped DVE tensor_reduce gives per-(b,k)
    norm^2.
  - sqrt on ACT, multiply by S, free-dim reduce -> [128,1] per core.
  - Host adds the 8x128 partials.
"""

import numpy as np

import concourse.bass as bass
import concourse.mybir as mybir
import concourse.tile as tile
from concourse import bacc
from concourse import bass_utils

N_ROWS = 500000
D = 128
B = 4096
K = 64
N_CORES = 8
B_PER_CORE = B // N_CORES      # 512
CHUNKS = 4
BC = B_PER_CORE // CHUNKS      # 128 b's per chunk (one per partition)
FP32 = mybir.dt.float32
I32 = mybir.dt.int32

# k-slots of the -P[i] broadcast written by ACT; the rest go to DVE.
KA = 40

_PROGRAM_CACHE = {}


def _build_program(repeat=1):
    nc = bacc.Bacc(
        "TRN2",
        debug=False,
        enable_asserts=False,
        target_bir_lowering=False,
        num_devices=N_CORES,
        dynamic_dma_scratch_size=65536,
    )

    P_d = nc.dram_tensor("P", [N_ROWS, D], FP32, kind="ExternalInput")
    idxI_d = nc.dram_tensor("idxI", [128, CHUNKS], I32, kind="ExternalInput")
    idxJ_d = nc.dram_tensor("idxJ", [128, CHUNKS * K], I32, kind="ExternalInput")
    S_d = nc.dram_tensor("S", [128, CHUNKS * K], FP32, kind="ExternalInput")
    out_d = nc.dram_tensor("out", [128, 1], FP32, kind="ExternalOutput")

    with tile.TileContext(nc) as tc:
        with (
            tc.tile_pool(name="persist", bufs=1) as pp,
            tc.tile_pool(name="work", bufs=2) as wp,
            tc.tile_pool(name="small", bufs=2) as sp,
        ):
            idxI_sb = pp.tile([128, CHUNKS], I32)
            nc.sync.dma_start(idxI_sb[:], idxI_d[:, :])
            idxJ_sb = pp.tile([128, CHUNKS * K], I32)
            nc.sync.dma_start(idxJ_sb[:], idxJ_d[:, :])
            S_sb = pp.tile([128, CHUNKS * K], FP32)
            nc.sync.dma_start(S_sb[:], S_d[:, :])

            # Gather the 512 P[i] rows (HW supports one index per partition
            # per indirect DMA): piAll[p, c, :] = P[idxI[p, c]]
            piAll = pp.tile([128, CHUNKS, D], FP32)
            for c in range(CHUNKS):
                nc.gpsimd.indirect_dma_start(
                    out=piAll[:, c, :],
                    out_offset=None,
                    in_=P_d[:, :],
                    in_offset=bass.IndirectOffsetOnAxis(
                        ap=idxI_sb[:, c : c + 1], axis=0
                    ),
                )

            norm2 = pp.tile([128, CHUNKS * K], FP32)

            for _rep in range(repeat):
              for c in range(CHUNKS):
                # big[p, k, :] <- P[j_{b,k}] for b = c*128+p.
                # HW consumes one index per partition per indirect DMA, and
                # CCE-accumulate doubles tx descriptors (Q7 desc-gen is the
                # bottleneck) -> plain per-k gathers, subtract on DVE.
                big = wp.tile([128, K, D], FP32)
                for k in range(K):
                    nc.gpsimd.indirect_dma_start(
                        out=big[:, k, :],
                        out_offset=None,
                        in_=P_d[:, :],
                        in_offset=bass.IndirectOffsetOnAxis(
                            ap=idxJ_sb[:, c * K + k : c * K + k + 1], axis=0
                        ),
                    )
                # diff on DVE (broadcast P[i_b] along k), square on ACT,
                # grouped free-dim reduce on DVE
                sq = wp.tile([128, K, D], FP32)
                nc.vector.tensor_tensor(
                    out=sq[:],
                    in0=big[:],
                    in1=piAll[:, c : c + 1, :].to_broadcast((128, K, D)),
                    op=mybir.AluOpType.subtract,
                )
                nc.scalar.square(sq[:], sq[:])
                nc.vector.reduce_sum(
                    norm2[:, c * K : (c + 1) * K],
                    sq[:],
                    axis=mybir.AxisListType.X,
                )

            # finals: sqrt -> *S -> free-dim reduce -> DRAM
            norms = pp.tile([128, CHUNKS * K], FP32)
            nc.scalar.sqrt(norms[:], norm2[:])
            weighted = pp.tile([128, CHUNKS * K], FP32)
            nc.vector.tensor_tensor(
                out=weighted[:], in0=norms[:], in1=S_sb[:], op=mybir.AluOpType.mult
            )
            res = pp.tile([128, 1], FP32)
            nc.vector.reduce_sum(res[:], weighted[:], axis=mybir.AxisListType.X)
            nc.sync.dma_start(out_d[:, :], res[:])

    nc.compile()
    return nc


def get_program(repeat=1):
    if repeat not in _PROGRAM_CACHE:
        _PROGRAM_CACHE[repeat] = _build_program(repeat)
    return _PROGRAM_CACHE[repeat]


def make_in_maps(P, i_indices, j_indices, S_vals):
    P = np.ascontiguousarray(np.asarray(P, dtype=np.float32))
    i_idx = np.asarray(i_indices).astype(np.int32)
    j_idx = np.asarray(j_indices).astype(np.int32)
    S = np.asarray(S_vals, dtype=np.float32)
    in_maps = []
    for core in range(N_CORES):
        b0 = core * B_PER_CORE
        i_c = i_idx[b0 : b0 + B_PER_CORE]            # [512]
        j_c = j_idx[b0 : b0 + B_PER_CORE]            # [512, 64]
        S_c = S[b0 : b0 + B_PER_CORE]                # [512, 64]
        idxI = np.ascontiguousarray(i_c.reshape(CHUNKS, BC).T)          # [128, 4]
        idxJ = np.ascontiguousarray(
            j_c.reshape(CHUNKS, BC, K).transpose(1, 0, 2).reshape(BC, CHUNKS * K)
        )
        S_arr = np.ascontiguousarray(
            S_c.reshape(CHUNKS, BC, K).transpose(1, 0, 2).reshape(BC, CHUNKS * K)
        )
        in_maps.append({"P": P, "idxI": idxI, "idxJ": idxJ, "S": S_arr})
    return in_maps


def run_hw(in_maps, trace=False, repeat=1):
    nc = get_program(repeat)
    return bass_utils.run_bass_kernel_spmd(
        nc,
        in_maps,
        core_ids=list(range(N_CORES)),
        trace=trace,
    )


def kernel(P, i_indices, j_indices, S_vals):
    in_maps = make_in_maps(P, i_indices, j_indices, S_vals)
    res = run_hw(in_maps, trace=False)
    total = 0.0
    for core in range(N_CORES):
        total += float(np.asarray(res.results[core]["out"], dtype=np.float64).sum())
    return np.float32(total)
